# revision 6
# baseline (speedup 1.0000x reference)
"""Class-balanced softmax cross-entropy loss on 8 Trainium2 NeuronCores.

Math: counts N_c over batch; w_c = (1-beta)/(1-beta^N_c) (0 if N_c=0);
loss = -sum w[t](logp[t]) / sum w[t] over valid pixels.

Fast path (used when all class weights are equal, which holds whenever every
class count N_c is large enough that beta^N_c underflows — always true for
this problem's 4.2M uniformly distributed pixels; verified exactly on host
via bincount): the weights cancel in the ratio, so
  loss = (sum_pix lse - sum_pix x[t]) / N_valid

fast3 (no-ignore) engine split per core:
  ACT : exp over all logits (16 batched instrs, the 65us/core floor) and
        per-chunk Ln(sumexp) read from PSUM with accum_out -> Σ lse.
  PE  : sumexp = Σ_c exp via identity-matmul PSUM accumulation (removes the
        DVE tree), and Σ x[t] = Σ products via ones-matmul into a single
        accumulating PSUM bank.
  DVE : per-class one-hot masks via tensor_scalar is_equal (4x mode) plus
        one in-place tensor_tensor product per class-group (2x mode); the
        old fused scalar_tensor_tensor path has no fast uops (1x) and was
        the 153us bottleneck.
Inputs host-cast: logits -> bf16 chunk-major [4*128, 19456], target -> f16.

Exact fallback path (any weight spread): original per-class A/B/N kernel.
"""

import numpy as np
import sys

for _p in ("/opt/trn_rl_repo",):
    if _p not in sys.path:
        sys.path.insert(0, _p)

import ml_dtypes
from concourse import bass, mybir
from concourse.bass_utils import run_bass_kernel_spmd

NCLASS = 19
BETA = 0.999
NCORES = 8
P = 128
COLS = 4096              # 512*1024 / 128
F = 1024                 # free-dim chunk
NCHUNK = COLS // F       # 4
EF = NCLASS * F          # 19456
GROUPS = [(0, 5), (5, 10), (10, 15), (15, 19)]
GW = 5 * F               # max group width in cols

f32 = mybir.dt.float32
f16 = mybir.dt.float16
bf16 = mybir.dt.bfloat16
i32 = mybir.dt.int32
AF = mybir.ActivationFunctionType
ALU = mybir.AluOpType


def _build_fast3():
    nc = bass.Bass()
    xp = nc.declare_dram_parameter("xp", [NCHUNK * P, EF], bf16, isOutput=False)
    tgt = nc.declare_dram_parameter("tgt", [P, COLS], f16, isOutput=False)
    ident_in = nc.declare_dram_parameter("ident", [P, P], f16, isOutput=False)
    out = nc.declare_dram_parameter("out", [1, 8], f32, isOutput=True)

    X2 = nc.alloc_sbuf_tensor("X2", [P, 2 * EF], bf16)
    E2 = nc.alloc_sbuf_tensor("E2", [P, 2 * EF], f16)
    T = nc.alloc_sbuf_tensor("T", [P, COLS], f16)
    M2 = nc.alloc_sbuf_tensor("M2", [P, 2 * GW], f16)
    junkL = nc.alloc_sbuf_tensor("junkL", [P, F], f16)
    ABN = nc.alloc_sbuf_tensor("ABN", [P, 8], f32)
    ident = nc.alloc_sbuf_tensor("identS", [P, P], f16)
    ones_g = nc.alloc_sbuf_tensor("ones_g", [P, 1], f16)
    ones_f = nc.alloc_sbuf_tensor("ones_f", [P, 1], f32)
    junkG = nc.alloc_sbuf_tensor("junkG", [1, 512], f16)
    res = nc.alloc_sbuf_tensor("res", [1, 8], f32)
    ps_s = nc.alloc_psum_tensor("ps_s", [P, 2 * F], f32)
    ps_g = nc.alloc_psum_tensor("ps_g", [1, 512], f32)
    ps2 = nc.alloc_psum_tensor("ps2", [1, 8], f32)

    NGRP = NCHUNK * len(GROUPS)  # 16

    # per-chunk X sub-DMA class splits (chunk 0 finer for pipeline fill) and
    # per-chunk exp instruction class splits (middle chunks batched; first and
    # last chunks grouped so PE can trail closely)
    DMA_SPLITS = [[(0, 2), (2, 5), (5, 10), (10, 15), (15, 19)]] + \
                 [[(0, 5), (5, 10), (10, 15), (15, 19)]] * 3
    EXP_SPLITS = [[(0, 2), (2, 5), (5, 10), (10, 15), (15, 19)],
                  [(0, 19)], [(0, 19)],
                  [(0, 5), (5, 10), (10, 15), (15, 19)]]
    # cumulative dma counts: dma_cum[k][c] = number of X dmas issued once
    # class c of chunk k has landed
    dma_done_at = []
    n = 0
    for k in range(NCHUNK):
        ends = {}
        for (lo, hi) in DMA_SPLITS[k]:
            n += 1
            ends[hi] = n
        dma_done_at.append(ends)

    def xdma_thr(k, hi):
        """s_x threshold for classes [0, hi) of chunk k to have landed."""
        return 16 * dma_done_at[k][hi]

    with (
        nc.Block() as block,
        nc.semaphore("s_t") as s_t,
        nc.semaphore("s_id") as s_id,
        nc.semaphore("s_x") as s_x,
        nc.semaphore("s_e") as s_e,      # counts classes exp'd: 19*k + hi
        nc.semaphore("s_tt") as s_tt,
        nc.semaphore("s_ps") as s_ps,
        nc.semaphore("s_pg") as s_pg,
        nc.semaphore("s_ln") as s_ln,
        nc.semaphore("s_gs") as s_gs,
        nc.semaphore("s_fin") as s_fin,
        nc.semaphore("s_out") as s_out,
        nc.allow_low_precision("f16 masks/products; f32 psum accumulation"),
    ):
        @block.sync
        def _(sp):
            # first two X slabs go out first so ACT can start early; T and
            # ident follow, then the rest of the X stream in order
            nx = 0
            for k in range(NCHUNK):
                h = k % 2
                for j, (lo, hi) in enumerate(DMA_SPLITS[k]):
                    if k >= 2:
                        # X half reused: chunk k-2's exp + products done
                        sp.wait_ge(s_e, 19 * (k - 2) + hi)
                        sp.wait_ge(s_tt, 4 * (k - 2) + min(4, (hi + 4) // 5))
                    sp.dma_start(
                        X2[:, h * EF + lo * F: h * EF + hi * F],
                        xp[k * P:(k + 1) * P, lo * F: hi * F],
                    ).then_inc(s_x, 16)
                    nx += 1
                    if nx == 2:
                        sp.dma_start(T[:], tgt[:, :]).then_inc(s_t, 16)
                        sp.dma_start(ident[:], ident_in[:, :]).then_inc(s_id, 16)

        @block.scalar
        def _(act):
            def ln_chunk(kk):
                hh = kk % 2
                act.wait_ge(s_ps, 4 * (kk + 1))
                act.activation(
                    junkL[:], ps_s[:, hh * F:(hh + 1) * F], AF.Ln,
                    accum_out=ABN[:, kk:kk + 1]).then_inc(s_ln, 1)

            for k in range(NCHUNK):
                h = k % 2
                for j, (lo, hi) in enumerate(EXP_SPLITS[k]):
                    act.wait_ge(s_x, xdma_thr(k, hi))
                    if k >= 2 and j == 0:
                        act.wait_ge(s_ps, 4 * (k - 1))   # E half reused
                    act.activation(
                        E2[:, h * EF + lo * F: h * EF + hi * F],
                        X2[:, h * EF + lo * F: h * EF + hi * F],
                        AF.Exp).then_inc(s_e, hi - lo)
                    if k >= 1 and j == min(1, len(EXP_SPLITS[k]) - 1):
                        ln_chunk(k - 1)
            ln_chunk(NCHUNK - 1)
            # tail: psums -> sbuf -> dram
            act.wait_ge(s_fin, 1)
            act.copy(res[:], ps2[:])
            act.dma_start(out[:, :], res[:]).then_inc(s_out, 16)
            act.wait_ge(s_out, 16)

        @block.vector
        def _(dve):
            dve.memset(ABN[:], 0.0)
            dve.memset(ones_g[:], 1.0)
            dve.memset(ones_f[:], 1.0)
            dve.wait_ge(s_t, 16)
            for k in range(NCHUNK):
                h = k % 2
                Tk = T[:, k * F:(k + 1) * F]
                for g, (lo, hi) in enumerate(GROUPS):
                    G = 4 * k + g
                    h2 = G % 2
                    W = (hi - lo) * F
                    Mg = M2[:, h2 * GW: h2 * GW + W]
                    if G >= 2:
                        dve.wait_ge(s_pg, G - 1)   # M half reused
                    for ci, c in enumerate(range(lo, hi)):
                        dve.tensor_scalar(
                            out=Mg[:, ci * F:(ci + 1) * F], in0=Tk,
                            scalar1=float(c), scalar2=None, op0=ALU.is_equal)
                    dve.wait_ge(s_x, xdma_thr(k, hi))
                    dve.tensor_tensor(
                        out=Mg[:], in0=Mg[:],
                        in1=X2[:, h * EF + lo * F: h * EF + hi * F],
                        op=ALU.mult).then_inc(s_tt, 1)
            # total gather sum: reduce ps_g into ABN[0, 4] (rows 1.. are 0)
            dve.wait_ge(s_pg, NGRP)
            dve.tensor_scalar(
                out=junkG[:], in0=ps_g[:, :], scalar1=1.0, scalar2=None,
                op0=ALU.mult, op1=ALU.add,
                accum_out=ABN[0:1, 4:5]).then_inc(s_gs, 1)

        @block.tensor
        def _(pe):
            pe.wait_ge(s_id, 16)
            first_g = True
            for k in range(NCHUNK):
                h = k % 2
                for g, (lo, hi) in enumerate(GROUPS):
                    G = 4 * k + g
                    h2 = G % 2
                    # sumexp accumulation for this group's classes
                    pe.wait_ge(s_e, 19 * k + hi)
                    if k >= 2 and g == 0:
                        pe.wait_ge(s_ln, k - 1)    # psum half reused
                    for c in range(lo, hi):
                        for half in range(2):
                            ins = pe.matmul(
                                ps_s[:, h * F + half * 512: h * F + (half + 1) * 512],
                                lhsT=ident[:, :],
                                rhs=E2[:, h * EF + c * F + half * 512:
                                        h * EF + c * F + (half + 1) * 512],
                                start=(c == 0), stop=(c == NCLASS - 1),
                                skip_group_check=True)
                            if c == hi - 1 and half == 1:
                                ins.then_inc(s_ps, 1)
                    # gather-sum accumulation of this group's products
                    pe.wait_ge(s_tt, G + 1)
                    n = (hi - lo) * 2
                    for j in range(n):
                        ins = pe.matmul(
                            ps_g[:, :], lhsT=ones_g[:],
                            rhs=M2[:, h2 * GW + j * 512: h2 * GW + (j + 1) * 512],
                            start=first_g,
                            stop=(G == NGRP - 1 and j == n - 1),
                            skip_group_check=True)
                        first_g = False
                        if j == n - 1:
                            ins.then_inc(s_pg, 1)
            pe.wait_ge(s_ln, NCHUNK)
            pe.wait_ge(s_gs, 1)
            pe.matmul(ps2[:], lhsT=ones_f[:], rhs=ABN[:],
                      start=True, stop=True,
                      skip_group_check=True).then_inc(s_fin, 1)

    return nc


def _build_fast():
    """Masked fast path (handles ignore pixels); original stt kernel."""
    nc = bass.Bass()
    logits = nc.declare_dram_parameter("logits", [NCLASS, P, COLS], bf16, isOutput=False)
    target = nc.declare_dram_parameter("target", [P, COLS], f32, isOutput=False)
    NG = NCHUNK * NCLASS
    ACC_COLS = NG + NCHUNK
    out = nc.declare_dram_parameter("out", [1, ACC_COLS], f32, isOutput=True)

    X2 = nc.alloc_sbuf_tensor("X2", [P, 2 * EF], bf16)
    E2 = nc.alloc_sbuf_tensor("E2", [P, 2 * EF], f16)
    T = nc.alloc_sbuf_tensor("T", [P, COLS], f32)
    L2 = nc.alloc_sbuf_tensor("L2", [P, 2 * F], f16)
    junk = nc.alloc_sbuf_tensor("junk", [P, F], f16)
    ABN = nc.alloc_sbuf_tensor("ABN", [P, ACC_COLS], f32)
    ones = nc.alloc_sbuf_tensor("ones", [P, 1], f32)
    res = nc.alloc_sbuf_tensor("res", [1, ACC_COLS], f32)
    ps = nc.alloc_psum_tensor("ps", [1, ACC_COLS], f32)

    with (
        nc.Block() as block,
        nc.semaphore("s_x") as s_x,
        nc.semaphore("s_t") as s_t,
        nc.semaphore("s_exp") as s_exp,
        nc.semaphore("s_tree") as s_tree,
        nc.semaphore("s_log") as s_log,
        nc.semaphore("s_gA") as s_gA,
        nc.semaphore("s_gB") as s_gB,
        nc.semaphore("s_mm") as s_mm,
        nc.semaphore("s_out") as s_out,
        nc.allow_low_precision("f16 tree-sum of exp; error ~0.1% on lse"),
    ):
        @block.sync
        def _(sp):
            sp.dma_start(T[:], target[:, :]).then_inc(s_t, 16)
            for k in range(NCHUNK):
                h = k % 2
                if k >= 2:
                    sp.wait_ge(s_gA, k - 1)
                sp.dma_start(
                    X2[:, h * EF:(h + 1) * EF].rearrange("p (c f) -> p c f", c=NCLASS),
                    logits[:, :, k * F:(k + 1) * F].rearrange("c p f -> p c f"),
                ).then_inc(s_x, 16)

        @block.scalar
        def _(act):
            for k in range(NCHUNK):
                h = k % 2
                act.wait_ge(s_x, 16 * (k + 1))
                E = E2[:, h * EF:(h + 1) * EF]
                X = X2[:, h * EF:(h + 1) * EF]
                for c in range(NCLASS):
                    ins = act.activation(
                        E[:, c * F:(c + 1) * F], X[:, c * F:(c + 1) * F], AF.Exp)
                    if c == NCLASS - 1:
                        ins.then_inc(s_exp, 1)
                act.wait_ge(s_tree, k + 1)
                if k >= 2:
                    act.wait_ge(s_gB, k - 1)   # L half reused
                act.activation(
                    L2[:, h * F:(h + 1) * F], E[:, 0:F], AF.Ln,
                ).then_inc(s_log, 1)
            act.wait_ge(s_mm, 1)
            act.copy(res[:], ps[:])
            act.dma_start(out[:, :], res[:]).then_inc(s_out, 16)
            act.wait_ge(s_out, 16)

        @block.vector
        def _(dve):
            dve.memset(ABN[:], 0.0)
            dve.memset(ones[:], 1.0)
            dve.wait_ge(s_t, 16)
            for k in range(NCHUNK):
                h = k % 2
                dve.wait_ge(s_exp, k + 1)
                E = E2[:, h * EF:(h + 1) * EF]
                dve.tensor_tensor(out=E[:, 0:3 * F], in0=E[:, 0:3 * F],
                                  in1=E[:, 16 * F:19 * F], op=ALU.add)
                dve.tensor_tensor(out=E[:, 0:8 * F], in0=E[:, 0:8 * F],
                                  in1=E[:, 8 * F:16 * F], op=ALU.add)
                dve.tensor_tensor(out=E[:, 0:4 * F], in0=E[:, 0:4 * F],
                                  in1=E[:, 4 * F:8 * F], op=ALU.add)
                dve.tensor_tensor(out=E[:, 0:2 * F], in0=E[:, 0:2 * F],
                                  in1=E[:, 2 * F:4 * F], op=ALU.add)
                dve.tensor_tensor(out=E[:, 0:F], in0=E[:, 0:F],
                                  in1=E[:, F:2 * F], op=ALU.add).then_inc(s_tree, 1)
                X = X2[:, h * EF:(h + 1) * EF]
                Tk = T[:, k * F:(k + 1) * F]
                for c in range(NCLASS):
                    ins = dve.scalar_tensor_tensor(
                        out=junk[:], in0=Tk, scalar=float(c),
                        in1=X[:, c * F:(c + 1) * F],
                        op0=ALU.is_equal, op1=ALU.mult,
                        accum_out=ABN[:, k * NCLASS + c: k * NCLASS + c + 1])
                    if c == NCLASS - 1:
                        ins.then_inc(s_gA, 1)
                dve.wait_ge(s_log, k + 1)
                dve.scalar_tensor_tensor(
                    out=junk[:], in0=Tk, scalar=-0.5,
                    in1=L2[:, h * F:(h + 1) * F],
                    op0=ALU.is_gt, op1=ALU.mult,
                    accum_out=ABN[:, NG + k: NG + k + 1]).then_inc(s_gB, 1)

        @block.tensor
        def _(pe):
            pe.wait_ge(s_gB, NCHUNK)
            pe.matmul(ps[:], lhsT=ones[:], rhs=ABN[:], start=True, stop=True
                      ).then_inc(s_mm, 1)

    return nc


def _build_exact():
    """Original per-class A/B/N kernel (correct for any weight pattern)."""
    nc = bass.Bass()
    F0 = 512
    NCH0 = COLS // F0
    SEC = NCH0 * NCLASS
    ACC0 = 3 * SEC
    logits = nc.declare_dram_parameter("logits", [NCLASS, P, COLS], f32, isOutput=False)
    target = nc.declare_dram_parameter("target", [P, COLS], i32, isOutput=False)
    out = nc.declare_dram_parameter("out", [1, ACC0], f32, isOutput=True)

    EF0 = NCLASS * F0
    X2 = nc.alloc_sbuf_tensor("X2", [P, 2 * EF0], f32)
    E2 = nc.alloc_sbuf_tensor("E2", [P, 2 * EF0], f32)
    Ti2 = nc.alloc_sbuf_tensor("Ti2", [P, 2 * F0], i32)
    Tf2 = nc.alloc_sbuf_tensor("Tf2", [P, 2 * F0], f32)
    S2 = nc.alloc_sbuf_tensor("S2", [P, 2 * F0], f32)
    L2 = nc.alloc_sbuf_tensor("L2", [P, 2 * F0], f32)
    junk = nc.alloc_sbuf_tensor("junk", [P, F0], f32)
    ABN = nc.alloc_sbuf_tensor("ABN", [P, ACC0], f32)
    ones = nc.alloc_sbuf_tensor("ones", [P, 1], f32)
    res = nc.alloc_sbuf_tensor("res", [1, ACC0], f32)
    ps = nc.alloc_psum_tensor("ps", [1, ACC0], f32)

    with (
        nc.Block() as block,
        nc.semaphore("sem_x") as sem_x,
        nc.semaphore("sem_t") as sem_t,
        nc.semaphore("sem_exp") as sem_exp,
        nc.semaphore("sem_red") as sem_red,
        nc.semaphore("sem_log") as sem_log,
        nc.semaphore("sem_done") as sem_done,
        nc.semaphore("sem_mm") as sem_mm,
        nc.semaphore("sem_out") as sem_out,
    ):
        @block.scalar
        def _(act):
            for k in range(NCH0):
                h = k % 2
                if k >= 2:
                    act.wait_ge(sem_done, k - 1)
                act.dma_start(
                    X2[:, h * EF0:(h + 1) * EF0].rearrange("p (c f) -> p c f", c=NCLASS),
                    logits[:, :, k * F0:(k + 1) * F0].rearrange("c p f -> p c f"),
                ).then_inc(sem_x, 16)
                act.dma_start(
                    Ti2[:, h * F0:(h + 1) * F0], target[:, k * F0:(k + 1) * F0],
                ).then_inc(sem_t, 16)
                act.wait_ge(sem_x, 16 * (k + 1))
                for c in range(NCLASS):
                    ins = act.activation(
                        E2[:, h * EF0 + c * F0: h * EF0 + (c + 1) * F0],
                        X2[:, h * EF0 + c * F0: h * EF0 + (c + 1) * F0], AF.Exp)
                    if c == NCLASS - 1:
                        ins.then_inc(sem_exp, 1)
                act.wait_ge(sem_red, k + 1)
                act.activation(
                    L2[:, h * F0:(h + 1) * F0], S2[:, h * F0:(h + 1) * F0], AF.Ln,
                ).then_inc(sem_log, 1)
            act.wait_ge(sem_mm, 1)
            act.copy(res[:], ps[:])
            act.dma_start(out[:, :], res[:]).then_inc(sem_out, 16)
            act.wait_ge(sem_out, 16)

        @block.vector
        def _(dve):
            dve.memset(ABN[:], 0.0)
            dve.memset(ones[:], 1.0)
            for k in range(NCH0):
                h = k % 2
                dve.wait_ge(sem_exp, k + 1)
                dve.tensor_reduce(
                    S2[:, h * F0:(h + 1) * F0],
                    E2[:, h * EF0:(h + 1) * EF0].rearrange("p (c f) -> p f c", c=NCLASS),
                    axis=mybir.AxisListType.X, op=ALU.add,
                ).then_inc(sem_red, 1)
                dve.wait_ge(sem_t, 16 * (k + 1))
                Ti = Tf2[:, h * F0:(h + 1) * F0]
                dve.tensor_copy(Ti[:], Ti2[:, h * F0:(h + 1) * F0])
                for c in range(NCLASS):
                    dve.scalar_tensor_tensor(
                        out=junk[:], in0=Ti[:], scalar=float(c),
                        in1=X2[:, h * EF0 + c * F0: h * EF0 + (c + 1) * F0],
                        op0=ALU.is_equal, op1=ALU.mult,
                        accum_out=ABN[:, 0 * SEC + k * NCLASS + c: 0 * SEC + k * NCLASS + c + 1])
                dve.wait_ge(sem_log, k + 1)
                LSE = L2[:, h * F0:(h + 1) * F0]
                for c in range(NCLASS):
                    dve.scalar_tensor_tensor(
                        out=junk[:], in0=Ti[:], scalar=float(c), in1=LSE[:],
                        op0=ALU.is_equal, op1=ALU.mult,
                        accum_out=ABN[:, 1 * SEC + k * NCLASS + c: 1 * SEC + k * NCLASS + c + 1])
                for c in range(NCLASS):
                    ins = dve.tensor_scalar(
                        out=junk[:], in0=Ti[:], scalar1=float(c), scalar2=None,
                        op0=ALU.is_equal, op1=ALU.add,
                        accum_out=ABN[:, 2 * SEC + k * NCLASS + c: 2 * SEC + k * NCLASS + c + 1])
                    if c == NCLASS - 1:
                        ins.then_inc(sem_done, 1)

        @block.tensor
        def _(pe):
            pe.wait_ge(sem_done, NCH0)
            pe.matmul(ps[:], lhsT=ones[:], rhs=ABN[:], start=True, stop=True).then_inc(sem_mm, 1)

    return nc


_CACHE = {}
_IDENT = np.eye(P, dtype=np.float16)


def _weights_and_counts(target):
    t = np.asarray(target).ravel()
    valid = (t >= 0) & (t < NCLASS)
    N = np.bincount(t[valid].astype(np.int64), minlength=NCLASS).astype(np.float64)
    with np.errstate(over="ignore"):
        w = np.where(N > 0, (1.0 - BETA) / (1.0 - np.power(np.float64(BETA), N)), 0.0)
    return w, N, int(valid.sum())


def _run_fast3(logits, target, trace=False):
    if "fast3" not in _CACHE:
        _CACHE["fast3"] = _build_fast3()
    nc = _CACHE["fast3"]
    lg = np.asarray(logits)
    tg = np.asarray(target)
    in_maps = []
    for i in range(NCORES):
        xp = np.ascontiguousarray(
            lg[i].reshape(NCLASS, P, NCHUNK, F).transpose(2, 1, 0, 3)
        ).reshape(NCHUNK * P, EF).astype(ml_dtypes.bfloat16)
        in_maps.append({
            "xp": xp,
            "tgt": tg[i].reshape(P, COLS).astype(np.float16),
            "ident": _IDENT,
        })
    return run_bass_kernel_spmd(nc, in_maps, core_ids=list(range(NCORES)), trace=trace)


def _combine_fast3(results, n_valid):
    G1 = 0.0
    G2 = 0.0
    for i in range(NCORES):
        r = results[i]["out"].astype(np.float64).reshape(8)
        G2 += r[0:4].sum()
        G1 += r[4]
    return np.float32((G2 - G1) / n_valid)


def _run_fast(logits, target, trace=False):
    if "fast" not in _CACHE:
        _CACHE["fast"] = _build_fast()
    nc = _CACHE["fast"]
    lg = np.asarray(logits)
    tg = np.asarray(target)
    in_maps = []
    for i in range(NCORES):
        in_maps.append({
            "logits": np.ascontiguousarray(
                lg[i].reshape(NCLASS, P, COLS)).astype(ml_dtypes.bfloat16),
            "target": np.ascontiguousarray(
                tg[i].reshape(P, COLS)).astype(np.float32),
        })
    return run_bass_kernel_spmd(nc, in_maps, core_ids=list(range(NCORES)), trace=trace)


def _combine_fast(results, n_valid):
    NG = NCHUNK * NCLASS
    G1 = 0.0
    G2 = 0.0
    for i in range(NCORES):
        r = results[i]["out"].astype(np.float64).reshape(NG + NCHUNK)
        G1 += r[:NG].sum()
        G2 += r[NG:].sum()
    return np.float32((G2 - G1) / n_valid)


def _run_exact(logits, target, trace=False):
    if "exact" not in _CACHE:
        _CACHE["exact"] = _build_exact()
    nc = _CACHE["exact"]
    in_maps = []
    for i in range(NCORES):
        in_maps.append({
            "logits": np.ascontiguousarray(
                np.asarray(logits)[i].reshape(NCLASS, P, COLS)),
            "target": np.ascontiguousarray(
                np.asarray(target)[i].reshape(P, COLS)),
        })
    return run_bass_kernel_spmd(nc, in_maps, core_ids=list(range(NCORES)), trace=trace)


def _combine_exact(results, w):
    F0 = 512
    NCH0 = COLS // F0
    A = np.zeros(NCLASS, np.float64)
    B = np.zeros(NCLASS, np.float64)
    N = np.zeros(NCLASS, np.float64)
    for i in range(NCORES):
        r = results[i]["out"].astype(np.float64).reshape(3, NCH0, NCLASS).sum(axis=1)
        A += r[0]
        B += r[1]
        N += r[2]
    num = float((w * (B - A)).sum())
    den = float((w * N).sum())
    return np.float32(num / den)


def kernel(logits, target):
    assert logits.shape == (NCORES, NCLASS, 512, 1024) and logits.dtype == np.float32
    assert target.shape == (NCORES, 512, 1024) and target.dtype == np.int32
    w, N, n_valid = _weights_and_counts(target)
    pos = w[N > 0]
    equal_w = pos.size > 0 and (pos.max() - pos.min()) <= 1e-9 * pos.mean()
    if equal_w:
        if n_valid == target.size:
            r = _run_fast3(logits, target)
            return _combine_fast3(r.results, n_valid)
        r = _run_fast(logits, target)
        return _combine_fast(r.results, n_valid)
    r = _run_exact(logits, target)
    return _combine_exact(r.results, w)


# revision 9
# speedup vs baseline: 1.1998x; 1.1998x over previous
"""Class-balanced softmax cross-entropy loss on 8 Trainium2 NeuronCores.

Math: counts N_c over batch; w_c = (1-beta)/(1-beta^N_c) (0 if N_c=0);
loss = -sum w[t](logp[t]) / sum w[t] over valid pixels.

Fast path (used when all class weights are equal, which holds whenever every
class count N_c is large enough that beta^N_c underflows — always true for
this problem's 4.2M uniformly distributed pixels; verified exactly on host
via bincount): the weights cancel in the ratio, so
  loss = (sum_pix lse - sum_pix x[t]) / N_valid

fast3 (no-ignore) engine split per core:
  ACT : exp over all logits (16 batched instrs, the 65us/core floor) and
        per-chunk Ln(sumexp) read from PSUM with accum_out -> Σ lse.
  PE  : sumexp = Σ_c exp via identity-matmul PSUM accumulation (removes the
        DVE tree), and Σ x[t] = Σ products via ones-matmul into a single
        accumulating PSUM bank.
  DVE : per-class one-hot masks via tensor_scalar is_equal (4x mode) plus
        one in-place tensor_tensor product per class-group (2x mode); the
        old fused scalar_tensor_tensor path has no fast uops (1x) and was
        the 153us bottleneck.
Inputs host-cast: logits -> bf16 chunk-major [4*128, 19456], target -> f16.

Exact fallback path (any weight spread): original per-class A/B/N kernel.
"""

import numpy as np
import sys

for _p in ("/opt/trn_rl_repo",):
    if _p not in sys.path:
        sys.path.insert(0, _p)

import ml_dtypes
from concourse import bass, mybir
from concourse.bass_utils import run_bass_kernel_spmd

NCLASS = 19
BETA = 0.999
NCORES = 8
P = 128
COLS = 4096              # 512*1024 / 128
F = 1024                 # free-dim chunk
NCHUNK = COLS // F       # 4
EF = NCLASS * F          # 19456
GROUPS = [(0, 5), (5, 10), (10, 15), (15, 19)]
GW = 5 * F               # max group width in cols

f32 = mybir.dt.float32
f16 = mybir.dt.float16
bf16 = mybir.dt.bfloat16
i32 = mybir.dt.int32
AF = mybir.ActivationFunctionType
ALU = mybir.AluOpType


def _build_fast3():
    nc = bass.Bass()
    xp = nc.declare_dram_parameter("xp", [NCHUNK * P, EF], bf16, isOutput=False)
    tgt = nc.declare_dram_parameter("tgt", [P, COLS], f16, isOutput=False)
    ident_in = nc.declare_dram_parameter("ident", [P, P], f16, isOutput=False)
    out = nc.declare_dram_parameter("out", [1, 8], f32, isOutput=True)

    X2 = nc.alloc_sbuf_tensor("X2", [P, 2 * EF], bf16)
    E2 = nc.alloc_sbuf_tensor("E2", [P, 2 * EF], f16)
    T = nc.alloc_sbuf_tensor("T", [P, COLS], f16)
    M2 = nc.alloc_sbuf_tensor("M2", [P, 2 * GW], f16)
    junkL = nc.alloc_sbuf_tensor("junkL", [P, F], f16)
    ABN = nc.alloc_sbuf_tensor("ABN", [P, 8], f32)
    ident = nc.alloc_sbuf_tensor("identS", [P, P], f16)
    ones_g = nc.alloc_sbuf_tensor("ones_g", [P, 1], f16)
    ones_f = nc.alloc_sbuf_tensor("ones_f", [P, 1], f32)
    junkG = nc.alloc_sbuf_tensor("junkG", [1, 512], f16)
    res = nc.alloc_sbuf_tensor("res", [1, 8], f32)
    ps_s = nc.alloc_psum_tensor("ps_s", [P, 2 * F], f32)
    ps_g = nc.alloc_psum_tensor("ps_g", [1, 512], f32)
    ps2 = nc.alloc_psum_tensor("ps2", [1, 8], f32)

    NGRP = NCHUNK * len(GROUPS)  # 16

    # per-chunk X sub-DMA class splits (chunk 0 finer for pipeline fill) and
    # per-chunk exp instruction class splits (middle chunks batched; first and
    # last chunks grouped so PE can trail closely)
    DMA_SPLITS = [[(0, 2), (2, 5), (5, 10), (10, 15), (15, 19)]] + \
                 [[(0, 5), (5, 10), (10, 15), (15, 19)]] * 3
    EXP_SPLITS = DMA_SPLITS
    # cumulative dma counts: dma_cum[k][c] = number of X dmas issued once
    # class c of chunk k has landed
    dma_done_at = []
    n = 0
    for k in range(NCHUNK):
        ends = {}
        for (lo, hi) in DMA_SPLITS[k]:
            n += 1
            ends[hi] = n
        dma_done_at.append(ends)

    def xdma_thr(k, hi):
        """s_x threshold for classes [0, hi) of chunk k to have landed."""
        return 16 * dma_done_at[k][hi]

    with (
        nc.Block() as block,
        nc.semaphore("s_t") as s_t,
        nc.semaphore("s_id") as s_id,
        nc.semaphore("s_x") as s_x,
        nc.semaphore("s_e") as s_e,      # counts classes exp'd: 19*k + hi
        nc.semaphore("s_tt") as s_tt,
        nc.semaphore("s_ps") as s_ps,
        nc.semaphore("s_pg") as s_pg,
        nc.semaphore("s_ln") as s_ln,
        nc.semaphore("s_gs") as s_gs,
        nc.semaphore("s_fin") as s_fin,
        nc.semaphore("s_out") as s_out,
        nc.allow_low_precision("f16 masks/products; f32 psum accumulation"),
    ):
        @block.sync
        def _(sp):
            # X slabs stream in order; T arrives per-chunk (small pieces
            # squeezed between slabs so DVE is fed without stalling ACT)
            for k in range(NCHUNK):
                h = k % 2
                for j, (lo, hi) in enumerate(DMA_SPLITS[k]):
                    if k >= 2:
                        # X half reused: chunk k-2's exp + products done
                        sp.wait_ge(s_e, 19 * (k - 2) + hi)
                        sp.wait_ge(s_tt, 4 * (k - 2) + min(4, (hi + 4) // 5))
                    sp.dma_start(
                        X2[:, h * EF + lo * F: h * EF + hi * F],
                        xp[k * P:(k + 1) * P, lo * F: hi * F],
                    ).then_inc(s_x, 16)
                    if k == 0 and j == 1:
                        sp.dma_start(T[:, 0:F], tgt[:, 0:F]).then_inc(s_t, 16)
                        sp.dma_start(ident[:], ident_in[:, :]).then_inc(s_id, 16)
                    elif k >= 1 and j == 0:
                        sp.dma_start(T[:, k * F:(k + 1) * F],
                                     tgt[:, k * F:(k + 1) * F]).then_inc(s_t, 16)

        @block.scalar
        def _(act):
            def ln_chunk(kk):
                hh = kk % 2
                act.wait_ge(s_ps, 4 * (kk + 1))
                act.activation(
                    junkL[:], ps_s[:, hh * F:(hh + 1) * F], AF.Ln,
                    accum_out=ABN[:, kk:kk + 1]).then_inc(s_ln, 1)

            for k in range(NCHUNK):
                h = k % 2
                for j, (lo, hi) in enumerate(EXP_SPLITS[k]):
                    act.wait_ge(s_x, xdma_thr(k, hi))
                    if k >= 2 and j == 0:
                        act.wait_ge(s_ps, 4 * (k - 1))   # E half reused
                    act.activation(
                        E2[:, h * EF + lo * F: h * EF + hi * F],
                        X2[:, h * EF + lo * F: h * EF + hi * F],
                        AF.Exp).then_inc(s_e, hi - lo)
                    if k >= 1 and j == min(1, len(EXP_SPLITS[k]) - 1):
                        ln_chunk(k - 1)
            ln_chunk(NCHUNK - 1)
            # tail: psums -> sbuf -> dram
            act.wait_ge(s_fin, 1)
            act.copy(res[:], ps2[:])
            act.dma_start(out[:, :], res[:]).then_inc(s_out, 16)
            act.wait_ge(s_out, 16)

        @block.vector
        def _(dve):
            dve.memset(ABN[:], 0.0)
            dve.memset(ones_g[:], 1.0)
            dve.memset(ones_f[:], 1.0)
            for k in range(NCHUNK):
                h = k % 2
                dve.wait_ge(s_t, 16 * (k + 1))
                Tk = T[:, k * F:(k + 1) * F]
                for g, (lo, hi) in enumerate(GROUPS):
                    G = 4 * k + g
                    h2 = G % 2
                    W = (hi - lo) * F
                    Mg = M2[:, h2 * GW: h2 * GW + W]
                    if G >= 2:
                        dve.wait_ge(s_pg, G - 1)   # M half reused
                    for ci, c in enumerate(range(lo, hi)):
                        dve.tensor_scalar(
                            out=Mg[:, ci * F:(ci + 1) * F], in0=Tk,
                            scalar1=float(c), scalar2=None, op0=ALU.is_equal)
                    dve.wait_ge(s_x, xdma_thr(k, hi))
                    dve.tensor_tensor(
                        out=Mg[:], in0=Mg[:],
                        in1=X2[:, h * EF + lo * F: h * EF + hi * F],
                        op=ALU.mult).then_inc(s_tt, 1)
            # total gather sum: reduce ps_g into ABN[0, 4] (rows 1.. are 0)
            dve.wait_ge(s_pg, NGRP)
            dve.tensor_scalar(
                out=junkG[:], in0=ps_g[:, :], scalar1=1.0, scalar2=None,
                op0=ALU.mult, op1=ALU.add,
                accum_out=ABN[0:1, 4:5]).then_inc(s_gs, 1)

        @block.tensor
        def _(pe):
            pe.wait_ge(s_id, 16)
            first_g = True
            for k in range(NCHUNK):
                h = k % 2
                for g, (lo, hi) in enumerate(GROUPS):
                    G = 4 * k + g
                    h2 = G % 2
                    # sumexp accumulation for this group's classes
                    pe.wait_ge(s_e, 19 * k + hi)
                    if k >= 2 and g == 0:
                        pe.wait_ge(s_ln, k - 1)    # psum half reused
                    for c in range(lo, hi):
                        for half in range(2):
                            ins = pe.matmul(
                                ps_s[:, h * F + half * 512: h * F + (half + 1) * 512],
                                lhsT=ident[:, :],
                                rhs=E2[:, h * EF + c * F + half * 512:
                                        h * EF + c * F + (half + 1) * 512],
                                start=(c == 0), stop=(c == NCLASS - 1),
                                skip_group_check=True)
                            if c == hi - 1 and half == 1:
                                ins.then_inc(s_ps, 1)
                    # gather-sum accumulation of this group's products
                    pe.wait_ge(s_tt, G + 1)
                    n = (hi - lo) * 2
                    for j in range(n):
                        ins = pe.matmul(
                            ps_g[:, :], lhsT=ones_g[:],
                            rhs=M2[:, h2 * GW + j * 512: h2 * GW + (j + 1) * 512],
                            start=first_g,
                            stop=(G == NGRP - 1 and j == n - 1),
                            skip_group_check=True)
                        first_g = False
                        if j == n - 1:
                            ins.then_inc(s_pg, 1)
            pe.wait_ge(s_ln, NCHUNK)
            pe.wait_ge(s_gs, 1)
            pe.matmul(ps2[:], lhsT=ones_f[:], rhs=ABN[:],
                      start=True, stop=True,
                      skip_group_check=True).then_inc(s_fin, 1)

    return nc


def _build_fast():
    """Masked fast path (handles ignore pixels); original stt kernel."""
    nc = bass.Bass()
    logits = nc.declare_dram_parameter("logits", [NCLASS, P, COLS], bf16, isOutput=False)
    target = nc.declare_dram_parameter("target", [P, COLS], f32, isOutput=False)
    NG = NCHUNK * NCLASS
    ACC_COLS = NG + NCHUNK
    out = nc.declare_dram_parameter("out", [1, ACC_COLS], f32, isOutput=True)

    X2 = nc.alloc_sbuf_tensor("X2", [P, 2 * EF], bf16)
    E2 = nc.alloc_sbuf_tensor("E2", [P, 2 * EF], f16)
    T = nc.alloc_sbuf_tensor("T", [P, COLS], f32)
    L2 = nc.alloc_sbuf_tensor("L2", [P, 2 * F], f16)
    junk = nc.alloc_sbuf_tensor("junk", [P, F], f16)
    ABN = nc.alloc_sbuf_tensor("ABN", [P, ACC_COLS], f32)
    ones = nc.alloc_sbuf_tensor("ones", [P, 1], f32)
    res = nc.alloc_sbuf_tensor("res", [1, ACC_COLS], f32)
    ps = nc.alloc_psum_tensor("ps", [1, ACC_COLS], f32)

    with (
        nc.Block() as block,
        nc.semaphore("s_x") as s_x,
        nc.semaphore("s_t") as s_t,
        nc.semaphore("s_exp") as s_exp,
        nc.semaphore("s_tree") as s_tree,
        nc.semaphore("s_log") as s_log,
        nc.semaphore("s_gA") as s_gA,
        nc.semaphore("s_gB") as s_gB,
        nc.semaphore("s_mm") as s_mm,
        nc.semaphore("s_out") as s_out,
        nc.allow_low_precision("f16 tree-sum of exp; error ~0.1% on lse"),
    ):
        @block.sync
        def _(sp):
            sp.dma_start(T[:], target[:, :]).then_inc(s_t, 16)
            for k in range(NCHUNK):
                h = k % 2
                if k >= 2:
                    sp.wait_ge(s_gA, k - 1)
                sp.dma_start(
                    X2[:, h * EF:(h + 1) * EF].rearrange("p (c f) -> p c f", c=NCLASS),
                    logits[:, :, k * F:(k + 1) * F].rearrange("c p f -> p c f"),
                ).then_inc(s_x, 16)

        @block.scalar
        def _(act):
            for k in range(NCHUNK):
                h = k % 2
                act.wait_ge(s_x, 16 * (k + 1))
                E = E2[:, h * EF:(h + 1) * EF]
                X = X2[:, h * EF:(h + 1) * EF]
                for c in range(NCLASS):
                    ins = act.activation(
                        E[:, c * F:(c + 1) * F], X[:, c * F:(c + 1) * F], AF.Exp)
                    if c == NCLASS - 1:
                        ins.then_inc(s_exp, 1)
                act.wait_ge(s_tree, k + 1)
                if k >= 2:
                    act.wait_ge(s_gB, k - 1)   # L half reused
                act.activation(
                    L2[:, h * F:(h + 1) * F], E[:, 0:F], AF.Ln,
                ).then_inc(s_log, 1)
            act.wait_ge(s_mm, 1)
            act.copy(res[:], ps[:])
            act.dma_start(out[:, :], res[:]).then_inc(s_out, 16)
            act.wait_ge(s_out, 16)

        @block.vector
        def _(dve):
            dve.memset(ABN[:], 0.0)
            dve.memset(ones[:], 1.0)
            dve.wait_ge(s_t, 16)
            for k in range(NCHUNK):
                h = k % 2
                dve.wait_ge(s_exp, k + 1)
                E = E2[:, h * EF:(h + 1) * EF]
                dve.tensor_tensor(out=E[:, 0:3 * F], in0=E[:, 0:3 * F],
                                  in1=E[:, 16 * F:19 * F], op=ALU.add)
                dve.tensor_tensor(out=E[:, 0:8 * F], in0=E[:, 0:8 * F],
                                  in1=E[:, 8 * F:16 * F], op=ALU.add)
                dve.tensor_tensor(out=E[:, 0:4 * F], in0=E[:, 0:4 * F],
                                  in1=E[:, 4 * F:8 * F], op=ALU.add)
                dve.tensor_tensor(out=E[:, 0:2 * F], in0=E[:, 0:2 * F],
                                  in1=E[:, 2 * F:4 * F], op=ALU.add)
                dve.tensor_tensor(out=E[:, 0:F], in0=E[:, 0:F],
                                  in1=E[:, F:2 * F], op=ALU.add).then_inc(s_tree, 1)
                X = X2[:, h * EF:(h + 1) * EF]
                Tk = T[:, k * F:(k + 1) * F]
                for c in range(NCLASS):
                    ins = dve.scalar_tensor_tensor(
                        out=junk[:], in0=Tk, scalar=float(c),
                        in1=X[:, c * F:(c + 1) * F],
                        op0=ALU.is_equal, op1=ALU.mult,
                        accum_out=ABN[:, k * NCLASS + c: k * NCLASS + c + 1])
                    if c == NCLASS - 1:
                        ins.then_inc(s_gA, 1)
                dve.wait_ge(s_log, k + 1)
                dve.scalar_tensor_tensor(
                    out=junk[:], in0=Tk, scalar=-0.5,
                    in1=L2[:, h * F:(h + 1) * F],
                    op0=ALU.is_gt, op1=ALU.mult,
                    accum_out=ABN[:, NG + k: NG + k + 1]).then_inc(s_gB, 1)

        @block.tensor
        def _(pe):
            pe.wait_ge(s_gB, NCHUNK)
            pe.matmul(ps[:], lhsT=ones[:], rhs=ABN[:], start=True, stop=True
                      ).then_inc(s_mm, 1)

    return nc


def _build_exact():
    """Original per-class A/B/N kernel (correct for any weight pattern)."""
    nc = bass.Bass()
    F0 = 512
    NCH0 = COLS // F0
    SEC = NCH0 * NCLASS
    ACC0 = 3 * SEC
    logits = nc.declare_dram_parameter("logits", [NCLASS, P, COLS], f32, isOutput=False)
    target = nc.declare_dram_parameter("target", [P, COLS], i32, isOutput=False)
    out = nc.declare_dram_parameter("out", [1, ACC0], f32, isOutput=True)

    EF0 = NCLASS * F0
    X2 = nc.alloc_sbuf_tensor("X2", [P, 2 * EF0], f32)
    E2 = nc.alloc_sbuf_tensor("E2", [P, 2 * EF0], f32)
    Ti2 = nc.alloc_sbuf_tensor("Ti2", [P, 2 * F0], i32)
    Tf2 = nc.alloc_sbuf_tensor("Tf2", [P, 2 * F0], f32)
    S2 = nc.alloc_sbuf_tensor("S2", [P, 2 * F0], f32)
    L2 = nc.alloc_sbuf_tensor("L2", [P, 2 * F0], f32)
    junk = nc.alloc_sbuf_tensor("junk", [P, F0], f32)
    ABN = nc.alloc_sbuf_tensor("ABN", [P, ACC0], f32)
    ones = nc.alloc_sbuf_tensor("ones", [P, 1], f32)
    res = nc.alloc_sbuf_tensor("res", [1, ACC0], f32)
    ps = nc.alloc_psum_tensor("ps", [1, ACC0], f32)

    with (
        nc.Block() as block,
        nc.semaphore("sem_x") as sem_x,
        nc.semaphore("sem_t") as sem_t,
        nc.semaphore("sem_exp") as sem_exp,
        nc.semaphore("sem_red") as sem_red,
        nc.semaphore("sem_log") as sem_log,
        nc.semaphore("sem_done") as sem_done,
        nc.semaphore("sem_mm") as sem_mm,
        nc.semaphore("sem_out") as sem_out,
    ):
        @block.scalar
        def _(act):
            for k in range(NCH0):
                h = k % 2
                if k >= 2:
                    act.wait_ge(sem_done, k - 1)
                act.dma_start(
                    X2[:, h * EF0:(h + 1) * EF0].rearrange("p (c f) -> p c f", c=NCLASS),
                    logits[:, :, k * F0:(k + 1) * F0].rearrange("c p f -> p c f"),
                ).then_inc(sem_x, 16)
                act.dma_start(
                    Ti2[:, h * F0:(h + 1) * F0], target[:, k * F0:(k + 1) * F0],
                ).then_inc(sem_t, 16)
                act.wait_ge(sem_x, 16 * (k + 1))
                for c in range(NCLASS):
                    ins = act.activation(
                        E2[:, h * EF0 + c * F0: h * EF0 + (c + 1) * F0],
                        X2[:, h * EF0 + c * F0: h * EF0 + (c + 1) * F0], AF.Exp)
                    if c == NCLASS - 1:
                        ins.then_inc(sem_exp, 1)
                act.wait_ge(sem_red, k + 1)
                act.activation(
                    L2[:, h * F0:(h + 1) * F0], S2[:, h * F0:(h + 1) * F0], AF.Ln,
                ).then_inc(sem_log, 1)
            act.wait_ge(sem_mm, 1)
            act.copy(res[:], ps[:])
            act.dma_start(out[:, :], res[:]).then_inc(sem_out, 16)
            act.wait_ge(sem_out, 16)

        @block.vector
        def _(dve):
            dve.memset(ABN[:], 0.0)
            dve.memset(ones[:], 1.0)
            for k in range(NCH0):
                h = k % 2
                dve.wait_ge(sem_exp, k + 1)
                dve.tensor_reduce(
                    S2[:, h * F0:(h + 1) * F0],
                    E2[:, h * EF0:(h + 1) * EF0].rearrange("p (c f) -> p f c", c=NCLASS),
                    axis=mybir.AxisListType.X, op=ALU.add,
                ).then_inc(sem_red, 1)
                dve.wait_ge(sem_t, 16 * (k + 1))
                Ti = Tf2[:, h * F0:(h + 1) * F0]
                dve.tensor_copy(Ti[:], Ti2[:, h * F0:(h + 1) * F0])
                for c in range(NCLASS):
                    dve.scalar_tensor_tensor(
                        out=junk[:], in0=Ti[:], scalar=float(c),
                        in1=X2[:, h * EF0 + c * F0: h * EF0 + (c + 1) * F0],
                        op0=ALU.is_equal, op1=ALU.mult,
                        accum_out=ABN[:, 0 * SEC + k * NCLASS + c: 0 * SEC + k * NCLASS + c + 1])
                dve.wait_ge(sem_log, k + 1)
                LSE = L2[:, h * F0:(h + 1) * F0]
                for c in range(NCLASS):
                    dve.scalar_tensor_tensor(
                        out=junk[:], in0=Ti[:], scalar=float(c), in1=LSE[:],
                        op0=ALU.is_equal, op1=ALU.mult,
                        accum_out=ABN[:, 1 * SEC + k * NCLASS + c: 1 * SEC + k * NCLASS + c + 1])
                for c in range(NCLASS):
                    ins = dve.tensor_scalar(
                        out=junk[:], in0=Ti[:], scalar1=float(c), scalar2=None,
                        op0=ALU.is_equal, op1=ALU.add,
                        accum_out=ABN[:, 2 * SEC + k * NCLASS + c: 2 * SEC + k * NCLASS + c + 1])
                    if c == NCLASS - 1:
                        ins.then_inc(sem_done, 1)

        @block.tensor
        def _(pe):
            pe.wait_ge(sem_done, NCH0)
            pe.matmul(ps[:], lhsT=ones[:], rhs=ABN[:], start=True, stop=True).then_inc(sem_mm, 1)

    return nc


_CACHE = {}
_IDENT = np.eye(P, dtype=np.float16)


def _weights_and_counts(target):
    t = np.asarray(target).ravel()
    valid = (t >= 0) & (t < NCLASS)
    N = np.bincount(t[valid].astype(np.int64), minlength=NCLASS).astype(np.float64)
    with np.errstate(over="ignore"):
        w = np.where(N > 0, (1.0 - BETA) / (1.0 - np.power(np.float64(BETA), N)), 0.0)
    return w, N, int(valid.sum())


def _run_fast3(logits, target, trace=False):
    if "fast3" not in _CACHE:
        _CACHE["fast3"] = _build_fast3()
    nc = _CACHE["fast3"]
    lg = np.asarray(logits)
    tg = np.asarray(target)
    in_maps = []
    for i in range(NCORES):
        xp = np.ascontiguousarray(
            lg[i].reshape(NCLASS, P, NCHUNK, F).transpose(2, 1, 0, 3)
        ).reshape(NCHUNK * P, EF).astype(ml_dtypes.bfloat16)
        in_maps.append({
            "xp": xp,
            "tgt": tg[i].reshape(P, COLS).astype(np.float16),
            "ident": _IDENT,
        })
    return run_bass_kernel_spmd(nc, in_maps, core_ids=list(range(NCORES)), trace=trace)


def _combine_fast3(results, n_valid):
    G1 = 0.0
    G2 = 0.0
    for i in range(NCORES):
        r = results[i]["out"].astype(np.float64).reshape(8)
        G2 += r[0:4].sum()
        G1 += r[4]
    return np.float32((G2 - G1) / n_valid)


def _run_fast(logits, target, trace=False):
    if "fast" not in _CACHE:
        _CACHE["fast"] = _build_fast()
    nc = _CACHE["fast"]
    lg = np.asarray(logits)
    tg = np.asarray(target)
    in_maps = []
    for i in range(NCORES):
        in_maps.append({
            "logits": np.ascontiguousarray(
                lg[i].reshape(NCLASS, P, COLS)).astype(ml_dtypes.bfloat16),
            "target": np.ascontiguousarray(
                tg[i].reshape(P, COLS)).astype(np.float32),
        })
    return run_bass_kernel_spmd(nc, in_maps, core_ids=list(range(NCORES)), trace=trace)


def _combine_fast(results, n_valid):
    NG = NCHUNK * NCLASS
    G1 = 0.0
    G2 = 0.0
    for i in range(NCORES):
        r = results[i]["out"].astype(np.float64).reshape(NG + NCHUNK)
        G1 += r[:NG].sum()
        G2 += r[NG:].sum()
    return np.float32((G2 - G1) / n_valid)


def _run_exact(logits, target, trace=False):
    if "exact" not in _CACHE:
        _CACHE["exact"] = _build_exact()
    nc = _CACHE["exact"]
    in_maps = []
    for i in range(NCORES):
        in_maps.append({
            "logits": np.ascontiguousarray(
                np.asarray(logits)[i].reshape(NCLASS, P, COLS)),
            "target": np.ascontiguousarray(
                np.asarray(target)[i].reshape(P, COLS)),
        })
    return run_bass_kernel_spmd(nc, in_maps, core_ids=list(range(NCORES)), trace=trace)


def _combine_exact(results, w):
    F0 = 512
    NCH0 = COLS // F0
    A = np.zeros(NCLASS, np.float64)
    B = np.zeros(NCLASS, np.float64)
    N = np.zeros(NCLASS, np.float64)
    for i in range(NCORES):
        r = results[i]["out"].astype(np.float64).reshape(3, NCH0, NCLASS).sum(axis=1)
        A += r[0]
        B += r[1]
        N += r[2]
    num = float((w * (B - A)).sum())
    den = float((w * N).sum())
    return np.float32(num / den)


def kernel(logits, target):
    assert logits.shape == (NCORES, NCLASS, 512, 1024) and logits.dtype == np.float32
    assert target.shape == (NCORES, 512, 1024) and target.dtype == np.int32
    w, N, n_valid = _weights_and_counts(target)
    pos = w[N > 0]
    equal_w = pos.size > 0 and (pos.max() - pos.min()) <= 1e-9 * pos.mean()
    if equal_w:
        if n_valid == target.size:
            r = _run_fast3(logits, target)
            return _combine_fast3(r.results, n_valid)
        r = _run_fast(logits, target)
        return _combine_fast(r.results, n_valid)
    r = _run_exact(logits, target)
    return _combine_exact(r.results, w)
